# revision 1
# baseline (speedup 1.0000x reference)
import numpy as np

H = 128
NCORES = 8
NCL, T, D = 1024, 64, 256
CPC = NCL // NCORES  # clusters per core


def _sigmoid(x):
    return 1.0 / (1.0 + np.exp(-x))


def _np_full(images, w_ih_l, w_hh_l, b_ih_l, b_hh_l, w_ih_h, w_hh_h, b_ih_h,
             b_hh_h, W_cf, b_cf, W_sf, b_sf, W_a1, b_a1, W_a2, b_a2):
    n_cluster = images.shape[0]
    h = np.zeros((n_cluster, H), np.float32)
    for t in range(images.shape[1]):
        x = images[:, t, :]
        gi = x @ w_ih_l.T + b_ih_l
        gh = h @ w_hh_l.T + b_hh_l
        r = _sigmoid(gi[:, :H] + gh[:, :H])
        z = _sigmoid(gi[:, H:2 * H] + gh[:, H:2 * H])
        n = np.tanh(gi[:, 2 * H:] + r * gh[:, 2 * H:])
        h = (1.0 - z) * n + z * h
    cluster_rep = h
    state_rep = _gru_high_host(cluster_rep, w_ih_h, w_hh_h, b_ih_h, b_hh_h)
    return _pairs_host(cluster_rep, state_rep, W_cf, b_cf, W_sf, b_sf,
                       W_a1, b_a1, W_a2, b_a2)


def _gru_high_host(cluster_rep, w_ih_h, w_hh_h, b_ih_h, b_hh_h):
    gi_all = cluster_rep @ w_ih_h.T + b_ih_h  # [n, 384]
    h = np.zeros((H,), np.float32)
    whhT = np.ascontiguousarray(w_hh_h.T)
    for k in range(cluster_rep.shape[0]):
        gh = h @ whhT + b_hh_h
        gi = gi_all[k]
        r = _sigmoid(gi[:H] + gh[:H])
        z = _sigmoid(gi[H:2 * H] + gh[H:2 * H])
        n = np.tanh(gi[2 * H:] + r * gh[2 * H:])
        h = (1.0 - z) * n + z * h
    return h


def _pair_prep(cluster_rep, state_rep, W_cf, b_cf, W_sf, b_sf, W_a1, b_a1, W_a2):
    c = np.maximum(cluster_rep @ W_cf.T + b_cf, 0.0)       # [n, 16]
    s = np.maximum(state_rep @ W_sf.T + b_sf, 0.0)         # [16]
    Ws, Wm = W_a1[:, :16], W_a1[:, 16:]
    base = Ws @ s + b_a1                                    # [32]
    u = c @ Wm.T                                            # [n, 32]
    ui = (u + base).astype(np.float32)
    return u.astype(np.float32), ui, W_a2[0].astype(np.float32)


def _pairs_host(cluster_rep, state_rep, W_cf, b_cf, W_sf, b_sf,
                W_a1, b_a1, W_a2, b_a2):
    u, ui, w2 = _pair_prep(cluster_rep, state_rep, W_cf, b_cf, W_sf, b_sf,
                           W_a1, b_a1, W_a2)
    ii, jj = np.tril_indices(cluster_rep.shape[0], k=-1)
    z = np.maximum(ui[ii] + u[jj], 0.0)
    q = z @ w2 + b_a2[0]
    q = q - q.max()
    e = np.exp(q)
    return (e / e.sum()).astype(np.float32)


def _softmax_from_qfull(q_full, b_a2):
    ii, jj = np.tril_indices(q_full.shape[0], k=-1)
    q = q_full[ii, jj].astype(np.float32) + np.float32(b_a2[0])
    q = q - q.max()
    e = np.exp(q)
    return (e / e.sum()).astype(np.float32)


# ---------------- device kernels ----------------

def _build_gru_low():
    import concourse.bass as bass
    import concourse.mybir as mybir
    from concourse import tile
    f32 = mybir.dt.float32
    AF = mybir.ActivationFunctionType

    nc = bass.Bass()
    xT = nc.dram_tensor("xT", [T, D, CPC], f32, kind="ExternalInput")
    wih = nc.dram_tensor("wihT", [D, 3 * H], f32, kind="ExternalInput")
    whh = nc.dram_tensor("whhT", [H, 3 * H], f32, kind="ExternalInput")
    br = nc.dram_tensor("br", [H, 1], f32, kind="ExternalInput")
    bz = nc.dram_tensor("bz", [H, 1], f32, kind="ExternalInput")
    bin_ = nc.dram_tensor("bin", [H, 1], f32, kind="ExternalInput")
    bhn = nc.dram_tensor("bhn", [H, 1], f32, kind="ExternalInput")
    hT_out = nc.dram_tensor("hT", [H, CPC], f32, kind="ExternalOutput")

    with tile.TileContext(nc) as tc:
        with (
            tc.tile_pool(name="const", bufs=1) as cp,
            tc.tile_pool(name="h", bufs=1) as hp,
            tc.tile_pool(name="x", bufs=4) as xp,
            tc.tile_pool(name="work", bufs=3) as wp,
            tc.tile_pool(name="ps", bufs=2, space="PSUM") as pp,
        ):
            wih0 = cp.tile([128, 3 * H], f32, tag="wih0")
            wih1 = cp.tile([128, 3 * H], f32, tag="wih1")
            whh_sb = cp.tile([128, 3 * H], f32, tag="whh")
            nc.sync.dma_start(wih0[:], wih[0:128, :])
            nc.sync.dma_start(wih1[:], wih[128:256, :])
            nc.sync.dma_start(whh_sb[:], whh[:])
            btiles = {}
            for nm, dr in (("br", br), ("bz", bz), ("bin", bin_), ("bhn", bhn)):
                bt = cp.tile([H, 1], f32, tag=nm)
                nc.sync.dma_start(bt[:], dr[:])
                btiles[nm] = bt
            hT = hp.tile([H, CPC], f32, tag="hT")
            nc.vector.memset(hT[:], 0.0)

            for t in range(T):
                x0 = xp.tile([128, CPC], f32, tag="x0")
                x1 = xp.tile([128, CPC], f32, tag="x1")
                nc.sync.dma_start(x0[:], xT[t, 0:128, :])
                nc.sync.dma_start(x1[:], xT[t, 128:256, :])

                pr = pp.tile([H, CPC], f32, tag="pr")
                pz = pp.tile([H, CPC], f32, tag="pz")
                pn1 = pp.tile([H, CPC], f32, tag="pn1")
                pn2 = pp.tile([H, CPC], f32, tag="pn2")
                # gi accumulation (2 K-chunks) + gh for r,z gates
                nc.tensor.matmul(pr[:], wih0[:, 0:H], x0[:], start=True, stop=False)
                nc.tensor.matmul(pr[:], wih1[:, 0:H], x1[:], start=False, stop=False)
                nc.tensor.matmul(pr[:], whh_sb[:, 0:H], hT[:], start=False, stop=True)
                nc.tensor.matmul(pz[:], wih0[:, H:2 * H], x0[:], start=True, stop=False)
                nc.tensor.matmul(pz[:], wih1[:, H:2 * H], x1[:], start=False, stop=False)
                nc.tensor.matmul(pz[:], whh_sb[:, H:2 * H], hT[:], start=False, stop=True)
                nc.tensor.matmul(pn1[:], wih0[:, 2 * H:], x0[:], start=True, stop=False)
                nc.tensor.matmul(pn1[:], wih1[:, 2 * H:], x1[:], start=False, stop=True)
                nc.tensor.matmul(pn2[:], whh_sb[:, 2 * H:], hT[:], start=True, stop=True)

                r = wp.tile([H, CPC], f32, tag="r")
                zt = wp.tile([H, CPC], f32, tag="z")
                n = wp.tile([H, CPC], f32, tag="n")
                tmp = wp.tile([H, CPC], f32, tag="tmp")
                nc.scalar.activation(r[:], pr[:], AF.Sigmoid, bias=btiles["br"][:])
                nc.scalar.activation(zt[:], pz[:], AF.Sigmoid, bias=btiles["bz"][:])
                nc.vector.tensor_scalar_add(tmp[:], pn2[:], btiles["bhn"][:])
                nc.vector.tensor_mul(tmp[:], tmp[:], r[:])
                nc.vector.tensor_add(tmp[:], tmp[:], pn1[:])
                nc.scalar.activation(n[:], tmp[:], AF.Tanh, bias=btiles["bin"][:])
                # h = n + z*(h - n)
                nc.vector.tensor_sub(tmp[:], hT[:], n[:])
                nc.vector.tensor_mul(tmp[:], tmp[:], zt[:])
                nc.vector.tensor_add(hT[:], n[:], tmp[:])

            nc.sync.dma_start(hT_out[:], hT[:])
    return nc


def _build_pairs():
    import concourse.bass as bass
    import concourse.mybir as mybir
    from concourse import tile
    f32 = mybir.dt.float32
    AF = mybir.ActivationFunctionType
    G = CPC // 4  # 32 groups of 4 i-rows per core
    NJ = NCL
    CH = 512

    nc = bass.Bass()
    uj = nc.dram_tensor("uj4", [128, NJ], f32, kind="ExternalInput")
    ui = nc.dram_tensor("ui4", [128, G], f32, kind="ExternalInput")
    w2b = nc.dram_tensor("w2b", [128, 4], f32, kind="ExternalInput")
    qout = nc.dram_tensor("q", [CPC, NJ], f32, kind="ExternalOutput")

    with tile.TileContext(nc) as tc:
        with (
            tc.tile_pool(name="const", bufs=1) as cp,
            tc.tile_pool(name="work", bufs=4) as wp,
            tc.tile_pool(name="ps", bufs=4, space="PSUM") as pp,
        ):
            uj_sb = cp.tile([128, NJ], f32, tag="uj")
            ui_sb = cp.tile([128, G], f32, tag="ui")
            w2_sb = cp.tile([128, 4], f32, tag="w2")
            nc.sync.dma_start(uj_sb[:], uj[:])
            nc.sync.dma_start(ui_sb[:], ui[:])
            nc.sync.dma_start(w2_sb[:], w2b[:])
            for g in range(G):
                for cidx in range(NJ // CH):
                    c0 = cidx * CH
                    zt = wp.tile([128, CH], f32, tag="z")
                    nc.vector.tensor_scalar_add(zt[:], uj_sb[:, c0:c0 + CH],
                                                ui_sb[:, g:g + 1])
                    rt = wp.tile([128, CH], f32, tag="r")
                    nc.scalar.activation(rt[:], zt[:], AF.Relu)
                    pq = pp.tile([4, CH], f32, tag="pq")
                    nc.tensor.matmul(pq[:], w2_sb[:], rt[:], start=True, stop=True)
                    qs = wp.tile([4, CH], f32, tag="q")
                    nc.vector.tensor_copy(qs[:], pq[:])
                    nc.sync.dma_start(qout[4 * g:4 * g + 4, c0:c0 + CH], qs[:])
    return nc


_CACHE = {}


def _device_kernel(images, w_ih_l, w_hh_l, b_ih_l, b_hh_l, w_ih_h, w_hh_h,
                   b_ih_h, b_hh_h, W_cf, b_cf, W_sf, b_sf, W_a1, b_a1,
                   W_a2, b_a2):
    from concourse.bass_utils import run_bass_kernel_spmd
    cores = list(range(NCORES))

    # ---- stage A: gru_low on device ----
    if "low" not in _CACHE:
        _CACHE["low"] = _build_gru_low()
    nc1 = _CACHE["low"]
    wihT = np.ascontiguousarray(w_ih_l.T)
    whhT = np.ascontiguousarray(w_hh_l.T)
    br = (b_ih_l[:H] + b_hh_l[:H]).reshape(H, 1).astype(np.float32)
    bz = (b_ih_l[H:2 * H] + b_hh_l[H:2 * H]).reshape(H, 1).astype(np.float32)
    bin_ = b_ih_l[2 * H:].reshape(H, 1).astype(np.float32)
    bhn = b_hh_l[2 * H:].reshape(H, 1).astype(np.float32)
    in_maps = []
    for k in cores:
        xT = np.ascontiguousarray(
            images[k * CPC:(k + 1) * CPC].transpose(1, 2, 0))
        in_maps.append({"xT": xT, "wihT": wihT, "whhT": whhT, "br": br,
                        "bz": bz, "bin": bin_, "bhn": bhn})
    res1 = run_bass_kernel_spmd(nc1, in_maps, cores)
    cluster_rep = np.concatenate(
        [res1.results[k]["hT"].T for k in cores], axis=0)  # [1024, 128]

    # ---- stage B: gru_high + projections on host ----
    state_rep = _gru_high_host(cluster_rep, w_ih_h, w_hh_h, b_ih_h, b_hh_h)
    u, ui, w2 = _pair_prep(cluster_rep, state_rep, W_cf, b_cf, W_sf, b_sf,
                           W_a1, b_a1, W_a2)

    # ---- stage C: pair sweep on device ----
    if "pairs" not in _CACHE:
        _CACHE["pairs"] = _build_pairs()
    nc2 = _CACHE["pairs"]
    UJ4 = np.ascontiguousarray(np.tile(u.T, (4, 1)))  # [128, 1024]
    w2blk = np.zeros((4, 32, 4), np.float32)
    for m in range(4):
        w2blk[m, :, m] = w2
    w2blk = w2blk.reshape(128, 4)
    in_maps2 = []
    for k in cores:
        uik = ui[k * CPC:(k + 1) * CPC]  # [128, 32]
        UI4 = np.ascontiguousarray(
            uik.reshape(32, 4, 32).transpose(1, 2, 0).reshape(128, 32))
        in_maps2.append({"uj4": UJ4, "ui4": UI4, "w2b": w2blk})
    res2 = run_bass_kernel_spmd(nc2, in_maps2, cores)
    q_full = np.concatenate([res2.results[k]["q"] for k in cores], axis=0)

    # ---- stage D: softmax on host ----
    return _softmax_from_qfull(q_full, b_a2)


def kernel(**inputs):
    inputs = {k: np.asarray(v, np.float32) for k, v in inputs.items()}
    try:
        return _device_kernel(**inputs)
    except Exception:
        import traceback
        traceback.print_exc()
        return _np_full(**inputs)

